# revision 3
# baseline (speedup 1.0000x reference)
"""MultiHeadCrossAttention (B=4, T=2048, C=1024, H=16, D=64) on 8 trn2 cores.

Sharding: core c -> (batch b=c//2, head-group g=c%2 of 8 heads).
Per core: fp16 projections with fused RoPE, row-tiled S^T, ACT exp softmax
(ones-augmented AV gives denominators), out-projection. Host sums the two
head-group partials per batch.
"""
import sys

sys.path.insert(0, "/opt/trn_rl_repo")

import numpy as np

B, T, C, H, D = 4, 2048, 1024, 16, 64
HPC = 8          # heads per core
NPAIR = 4        # head pairs per core
NCORES = 8
THETA = 10000.0

_cache = {}


def _rope_tables_np(t_len, d):
    exps = np.arange(1, d + 1, 2, dtype=np.float64)[: d // 2] / d
    w = 1.0 / (THETA ** exps)                      # [32]
    t = np.arange(1, t_len + 1, dtype=np.float64)
    ang = t[:, None] * w[None, :]                  # [T, 32]
    return np.sin(ang), np.cos(ang)


def _build_program():
    import concourse.bass as bass
    import concourse.mybir as mybir
    import concourse.tile as tile
    from concourse import bacc

    F32 = mybir.dt.float32
    FP16 = mybir.dt.float16
    AF = mybir.ActivationFunctionType

    nc = bacc.Bacc("TRN2", target_bir_lowering=False, debug=False)

    xq_d = nc.declare_dram_parameter("xq", [128, 8, T], FP16, isOutput=False)
    xkv_d = nc.declare_dram_parameter("xkv", [128, 8, T], FP16, isOutput=False)
    wq_d = nc.declare_dram_parameter("wq", [128, 8, NPAIR, 128], FP16, isOutput=False)
    wk_d = nc.declare_dram_parameter("wk", [128, 8, NPAIR, 128], FP16, isOutput=False)
    wv_d = nc.declare_dram_parameter("wv", [128, 8, 512], FP16, isOutput=False)
    wc_d = nc.declare_dram_parameter("wc", [128, NPAIR, C], FP16, isOutput=False)
    sins_d = nc.declare_dram_parameter("sins", [128, T], FP16, isOutput=False)
    cosa_d = nc.declare_dram_parameter("cosa", [128, T], FP16, isOutput=False)
    out_d = nc.declare_dram_parameter("out", [T, C], FP16, isOutput=True)

    with tile.TileContext(nc) as tc:
        with (
            tc.tile_pool(name="persist", bufs=1) as pp,
            tc.tile_pool(name="rope", bufs=2) as rp,
            tc.tile_pool(name="xstream", bufs=2) as xsp,
            tc.tile_pool(name="norm", bufs=3) as np_,
            tc.tile_pool(name="ystage", bufs=3) as yp,
            tc.tile_pool(name="pt_pool", bufs=3) as ptp,
            tc.tile_pool(name="psum", bufs=1, space="PSUM") as ps,
        ):
            wq = pp.tile([128, 8, NPAIR, 128], FP16)
            wk = pp.tile([128, 8, NPAIR, 128], FP16)
            wv = pp.tile([128, 8, 512], FP16)
            wc = pp.tile([128, NPAIR, C], FP16)
            sins = pp.tile([128, T], FP16)
            cosa = pp.tile([128, T], FP16)
            qt = pp.tile([128, NPAIR, T], FP16)
            kt = pp.tile([128, NPAIR, T], FP16)
            vt = pp.tile([128, 16, 8 * 65], FP16)
            ytn = pp.tile([128, NPAIR, T], FP16)
            onescol = pp.tile([1, 64], FP16)

            nc.sync.dma_start(wq[:], wq_d[:])
            nc.sync.dma_start(wk[:], wk_d[:])
            nc.sync.dma_start(wv[:], wv_d[:])
            nc.sync.dma_start(wc[:], wc_d[:])
            nc.sync.dma_start(sins[:], sins_d[:])
            nc.sync.dma_start(cosa[:], cosa_d[:])
            nc.vector.memset(vt[:], 1.0)
            nc.vector.memset(onescol[:], 1.0)

            def proj_rope_chunk(p, x_dram, w_tile, dst, tci):
                # dst[:, p, tci-chunk] = rope(x^T @ W_p), layout per 32 rows:
                # [A-x1 | A-x2 | B-x1 | B-x2]
                xs = xsp.tile([128, 8, 1024], FP16, tag="xs", bufs=2, name="xs")
                nc.sync.dma_start(xs[:], x_dram[:, :, bass.ds(tci * 1024, 1024)])
                psq = ps.tile([128, 2, 512], F32, tag="mix", bufs=1, name="psq")
                for cc in range(8):
                    for nh in range(2):
                        nc.tensor.matmul(
                            psq[:, nh, :],
                            w_tile[:, cc, p, :],
                            xs[:, cc, bass.ds(nh * 512, 512)],
                            start=(cc == 0),
                            stop=(cc == 7),
                        )
                tsl = bass.ds(tci * 1024, 1024)
                raw = rp.tile([128, 1024], FP16, tag="raw", bufs=2, name="raw")
                nc.vector.tensor_copy(raw[:], psq[:].rearrange("p a b -> p (a b)"))
                swp = rp.tile([128, 1024], FP16, tag="swp", bufs=2, name="swp")
                nc.sync.dma_start(swp[32:64, :], raw[0:32, :])
                nc.sync.dma_start(swp[0:32, :], raw[32:64, :])
                nc.sync.dma_start(swp[96:128, :], raw[64:96, :])
                nc.sync.dma_start(swp[64:96, :], raw[96:128, :])
                tt = rp.tile([128, 1024], FP16, tag="tt", bufs=2, name="tt")
                nc.vector.tensor_mul(tt[:], swp[:], sins[:, tsl])
                uu = rp.tile([128, 1024], FP16, tag="uu", bufs=2, name="uu")
                nc.vector.tensor_mul(uu[:], raw[:], cosa[:, tsl])
                nc.vector.tensor_add(dst[:, p, tsl], uu[:], tt[:])

            def proj_rope(p):
                for tci in range(2):
                    proj_rope_chunk(p, xq_d, wq, qt, tci)
                    proj_rope_chunk(p, xkv_d, wk, kt, tci)

            def proj_v():
                for tt_ in range(16):
                    xv = xsp.tile([128, 8, 128], FP16, tag="xv", bufs=2, name="xv")
                    nc.sync.dma_start(xv[:], xkv_d[:, :, bass.ds(tt_ * 128, 128)])
                    psv = ps.tile([128, 2, 512], F32, tag="mix", bufs=1, name="psv")
                    for cc in range(8):
                        nc.tensor.matmul(
                            psv[:, 0, :],
                            xv[:, cc, :],
                            wv[:, cc, :],
                            start=(cc == 0),
                            stop=(cc == 7),
                        )
                    nc.scalar.copy(
                        vt[:, tt_, :].rearrange("p (h d) -> p h d", h=8)[:, :, 0:64],
                        psv[:, 0, :].rearrange("p (h d) -> p h d", h=8),
                    )

            def attention(p, qc):
                qsl = bass.ds(qc * 512, 512)
                ytuA = ps.tile([65, 512], F32, tag="ytu", bufs=2, name="ytuA")
                ytuB = ps.tile([65, 512], F32, tag="ytu", bufs=2, name="ytuB")
                for g in range(8):
                    stA = ps.tile([128, 2, 512], F32, tag="st", bufs=2, name="stA")
                    stB = ps.tile([128, 2, 512], F32, tag="st", bufs=2, name="stB")
                    for j in range(2):
                        ksl = bass.ds((2 * g + j) * 128, 128)
                        nc.tensor.matmul(
                            stA[:, j, :], kt[0:64, p, ksl], qt[0:64, p, qsl],
                            start=True, stop=True,
                        )
                        nc.tensor.matmul(
                            stB[:, j, :], kt[64:128, p, ksl], qt[64:128, p, qsl],
                            start=True, stop=True,
                        )
                    ptA = ptp.tile([128, 2, 512], FP16, tag="pt", bufs=4, name="ptA")
                    ptB = ptp.tile([128, 2, 512], FP16, tag="pt", bufs=4, name="ptB")
                    nc.scalar.activation(
                        ptA[:].rearrange("p a b -> p (a b)"),
                        stA[:].rearrange("p a b -> p (a b)"),
                        AF.Exp, scale=0.125,
                    )
                    nc.scalar.activation(
                        ptB[:].rearrange("p a b -> p (a b)"),
                        stB[:].rearrange("p a b -> p (a b)"),
                        AF.Exp, scale=0.125,
                    )
                    for j in range(2):
                        k_ = 2 * g + j
                        nc.tensor.matmul(
                            ytuA[:], vt[:, k_, bass.ds((2 * p) * 65, 65)], ptA[:, j, :],
                            start=(k_ == 0), stop=(k_ == 15), skip_group_check=True,
                        )
                        nc.tensor.matmul(
                            ytuB[:], vt[:, k_, bass.ds((2 * p + 1) * 65, 65)],
                            ptB[:, j, :],
                            start=(k_ == 0), stop=(k_ == 15), skip_group_check=True,
                        )
                # normalize: recip of denominator row, PE broadcast, multiply
                recA = np_.tile([1, 512], F32, tag="rec32", bufs=2, name="recA")
                recB = np_.tile([1, 512], F32, tag="rec32", bufs=2, name="recB")
                nc.vector.reciprocal(recA[:], ytuA[64:65, :])
                nc.vector.reciprocal(recB[:], ytuB[64:65, :])
                recA16 = np_.tile([1, 512], FP16, tag="rec16", bufs=2, name="recA16")
                recB16 = np_.tile([1, 512], FP16, tag="rec16", bufs=2, name="recB16")
                nc.vector.tensor_copy(recA16[:], recA[:])
                nc.vector.tensor_copy(recB16[:], recB[:])
                denb = ps.tile([128, 2, 512], F32, tag="mix", bufs=1, name="denb")
                nc.tensor.matmul(denb[0:64, 0, :], onescol[:], recA16[:],
                                 start=True, stop=True)
                nc.tensor.matmul(denb[0:64, 1, :], onescol[:], recB16[:],
                                 start=True, stop=True)
                denbSA = np_.tile([64, 512], F32, tag="denbS", bufs=2, name="denbSA")
                denbSB = np_.tile([64, 512], F32, tag="denbS", bufs=2, name="denbSB")
                nc.vector.tensor_copy(denbSA[:], denb[0:64, 0, :])
                nc.vector.tensor_copy(denbSB[:], denb[0:64, 1, :])
                nc.vector.tensor_mul(ytn[0:64, p, qsl], ytuA[0:64, :], denbSA[:])
                nc.vector.tensor_mul(ytn[64:128, p, qsl], ytuB[0:64, :], denbSB[:])

            def outproj(qc):
                for ti in range(4):
                    tt_ = qc * 4 + ti
                    psy = ps.tile([128, 2, 512], F32, tag="mix", bufs=1, name="psy")
                    for ch in range(2):
                        for p in range(NPAIR):
                            nc.tensor.matmul(
                                psy[:, ch, :],
                                ytn[:, p, bass.ds(tt_ * 128, 128)],
                                wc[:, p, bass.ds(ch * 512, 512)],
                                start=(p == 0), stop=(p == NPAIR - 1),
                            )
                    ystg = yp.tile([128, 1024], FP16, tag="ystg", bufs=3, name="ystg")
                    nc.vector.tensor_copy(
                        ystg[:], psy[:].rearrange("p a b -> p (a b)")
                    )
                    nc.sync.dma_start(out_d[bass.ds(tt_ * 128, 128), :], ystg[:])

            # Emission order = per-engine program order; interleave next pair's
            # projection chunks into the attention loop to fill PE slack.
            proj_rope(0)
            proj_v()
            for p in range(NPAIR):
                for qc in range(4):
                    attention(p, qc)
                    if p < NPAIR - 1:
                        nxt = [(xq_d, wq, qt), (xkv_d, wk, kt)]
                        src = nxt[qc % 2]
                        proj_rope_chunk(p + 1, src[0], src[1], src[2], qc // 2)
                    else:
                        outproj(qc)

    nc.compile()
    return nc


def _host_prep(x_q, x_kv, W_q, W_kv, W_c):
    f16 = np.float16
    sin64, cos64 = _rope_tables_np(T, D)          # [T, 32] float64
    sinT = sin64.T                                 # [32, T]
    cosT = cos64.T
    # quarter layouts: rows [x1(32) | x2(32) | x1 | x2]
    sins = np.concatenate([-sinT, sinT, -sinT, sinT], 0).astype(f16)  # [128, T]
    cosa = np.concatenate([cosT, cosT, cosT, cosT], 0).astype(f16)

    in_maps = []
    for c in range(NCORES):
        b, g = c // 2, c % 2
        heads = np.arange(g * HPC, (g + 1) * HPC)

        xq_t = np.ascontiguousarray(
            x_q[b].T.reshape(8, 128, T).transpose(1, 0, 2)
        ).astype(f16)                              # [128cin, 8cc, T]
        xkv_t = np.ascontiguousarray(
            x_kv[b].T.reshape(8, 128, T).transpose(1, 0, 2)
        ).astype(f16)

        def qk_weights(w):                         # w [C, C] cols by head
            cols = []
            for p in range(NPAIR):
                for h in (heads[2 * p], heads[2 * p + 1]):
                    base = h * D
                    cols.append(np.arange(base, base + D, 2))      # x1: even d
                    cols.append(np.arange(base + 1, base + D, 2))  # x2: odd d
            idx = np.concatenate(cols)             # [512]
            wp = w[:, idx]                         # [1024, 512]
            return np.ascontiguousarray(
                wp.reshape(8, 128, NPAIR, 128).transpose(1, 0, 2, 3)
            ).astype(f16)                          # [128cin, 8cc, 4p, 128]

        wqh = qk_weights(W_q)
        wkh = qk_weights(W_kv[:, :C])

        vcols = np.concatenate([np.arange(h * D, (h + 1) * D) for h in heads])
        wvh = np.ascontiguousarray(
            W_kv[:, C:][:, vcols].reshape(8, 128, 512).transpose(1, 0, 2)
        ).astype(f16)                              # [128, 8, 512]

        rows = np.concatenate([np.arange(h * D, (h + 1) * D) for h in heads])
        wch = np.ascontiguousarray(
            W_c[rows, :].reshape(NPAIR, 128, C).transpose(1, 0, 2)
        ).astype(f16)                              # [128, 4p, C]

        in_maps.append({
            "xq": xq_t, "xkv": xkv_t, "wq": wqh, "wk": wkh,
            "wv": wvh, "wc": wch, "sins": sins, "cosa": cosa,
        })
    return in_maps


def _reference_np(x_q, x_kv, q_tok_mask, kv_tok_mask, W_q, W_kv, W_c):
    # exact numpy fallback (masks not all ones)
    out = np.zeros((B, T, C), np.float32)
    sin64, cos64 = _rope_tables_np(T, D)

    def rot(x):                                    # x [H, T, D]
        x1, x2 = x[..., ::2], x[..., 1::2]
        c = cos64[None, :, :]
        s = sin64[None, :, :]
        x1p = x1 * c - x2 * s
        x2p = x1 * s + x2 * c
        o = np.empty_like(x)
        o[..., ::2], o[..., 1::2] = x1p, x2p
        return o

    for b in range(B):
        q = x_q[b].astype(np.float64) @ W_q.astype(np.float64)
        kv = x_kv[b].astype(np.float64) @ W_kv.astype(np.float64)
        k, v = kv[:, :C], kv[:, C:]
        q = rot(q.reshape(T, H, D).transpose(1, 0, 2))
        k = rot(k.reshape(T, H, D).transpose(1, 0, 2))
        v = v.reshape(T, H, D).transpose(1, 0, 2)
        att = q @ k.transpose(0, 2, 1) / np.sqrt(D)
        mask = (q_tok_mask[b][:, None] & kv_tok_mask[b][None, :])[None]
        att = np.where(mask, att, -1e9)
        att = att - att.max(-1, keepdims=True)
        att = np.exp(att)
        att /= att.sum(-1, keepdims=True)
        y = (att @ v).transpose(1, 0, 2).reshape(T, C)
        out[b] = (y @ W_c.astype(np.float64)).astype(np.float32)
    return out


def kernel(x_q, x_kv, q_tok_mask, kv_tok_mask, W_q, W_kv, W_c):
    x_q = np.asarray(x_q, np.float32)
    x_kv = np.asarray(x_kv, np.float32)
    W_q = np.asarray(W_q, np.float32)
    W_kv = np.asarray(W_kv, np.float32)
    W_c = np.asarray(W_c, np.float32)
    q_tok_mask = np.asarray(q_tok_mask)
    kv_tok_mask = np.asarray(kv_tok_mask)

    if not (q_tok_mask.all() and kv_tok_mask.all()):
        return _reference_np(x_q, x_kv, q_tok_mask, kv_tok_mask, W_q, W_kv, W_c)

    from concourse.bass_utils import run_bass_kernel_spmd

    if "nc" not in _cache:
        _cache["nc"] = _build_program()
    nc = _cache["nc"]

    in_maps = _host_prep(x_q, x_kv, W_q, W_kv, W_c)
    res = run_bass_kernel_spmd(nc, in_maps, list(range(NCORES)))
    outs = [r["out"].astype(np.float32) for r in res.results]

    y = np.empty((B, T, C), np.float32)
    for b in range(B):
        y[b] = outs[2 * b] + outs[2 * b + 1]
    return y
